# revision 18
# baseline (speedup 1.0000x reference)
"""BinaryLinear (8192x4096 @ 4096x4096 binarized) on 8 TRN2 NeuronCores.

Strategy (tensor-parallel, column sharding per out_features):
  - Shard W/alpha/b along out_features: each core gets 512 output channels.
  - Replicate x (host pre-transposed to [in_f, n_rows] so the contraction
    dim lands on SBUF partitions without any device-side transpose).
  - Per core: out_shard[n, o] = sum_k xT[k, n] * bwT[k, o] + b[o], where
    bw = sign(W) * alpha is computed on device in fp32 (exact match of
    jnp.where(W >= 0, 1, -1) * alpha), then cast to the matmul dtype.
  - Host gathers the 8 [8192, 512] shards with a concatenate on axis 1.

Matmul layout per core:
  lhsT = x tile [K=128, M=128] (stationary), rhs = bwT tile [K=128, N=512]
  (moving), accumulating over 32 K-tiles into a [128, 512] PSUM bank.

Variants:
  f32    - full-precision fp32 matmul (4 cyc/row), reference-grade
  f32r   - fp32 storage, reduced-precision PE mode (~1e-4 rel err)
  bf16   - x shipped as bf16 (halves x DMA), weights binarized on device
           then cast to bf16 (~2e-3 rel err, fastest)
"""

import os
import sys

sys.path.insert(0, "/opt/trn_rl_repo")

import numpy as np

from concourse import bacc, bass, mybir
import concourse.tile as tile
from concourse.bass_utils import run_bass_kernel_spmd

N_ROWS = 8192
IN_F = 4096
OUT_F = 4096
N_CORES = 8
O_SHARD = OUT_F // N_CORES  # 512

P = 128

VARIANT = "bf16"  # f32 | f32r | bf16


def build_nc(
    n_rows=N_ROWS,
    in_f=IN_F,
    o_shard=O_SHARD,
    variant=VARIANT,
    n_chunk=None,
    x_bufs=16,
):
    """Build the per-core Bass graph (same program on all cores, SPMD)."""
    f32 = mybir.dt.float32
    if variant == "f32":
        x_dt = mm_dt = f32
    elif variant == "f32r":
        x_dt = mm_dt = mybir.dt.float32r
    elif variant == "bf16":
        x_dt = mm_dt = mybir.dt.bfloat16
    else:
        raise ValueError(variant)
    if n_chunk is None:
        n_chunk = 512

    assert in_f % P == 0 and n_rows % n_chunk == 0 and n_chunk % P == 0
    assert o_shard <= 512  # one PSUM bank per psum tile (fp32 out)
    KO = in_f // P
    NCH = n_rows // n_chunk
    NS = n_chunk // P
    assert NS * 2 <= 16  # psum tags * bufs fit in 8 banks w/ bufs>=1

    nc = bacc.Bacc("TRN2", target_bir_lowering=False)

    # f32r is fp32 storage; type the whole W/alpha producer chain f32r so the
    # BIR verifier's checkMatmultFP32r accepts the matmul inputs.
    w_in_dt = mm_dt if variant == "f32r" else f32
    xT = nc.declare_dram_parameter("xT", [in_f, n_rows], x_dt, isOutput=False)
    WT = nc.declare_dram_parameter("WT", [in_f, o_shard], w_in_dt, isOutput=False)
    a_rep = nc.declare_dram_parameter("a_rep", [P, o_shard], w_in_dt, isOutput=False)
    b_rep = nc.declare_dram_parameter("b_rep", [P, o_shard], f32, isOutput=False)
    out = nc.declare_dram_parameter("out", [n_rows, o_shard], f32, isOutput=True)

    xT_t = xT[:].rearrange("(ko p) n -> ko p n", p=P)
    WT_t = WT[:].rearrange("(ko p) o -> p ko o", p=P)

    psum_bufs = 2 if NS * 2 <= 8 else 1

    with tile.TileContext(nc) as tc:
        with (
            tc.tile_pool(name="consts", bufs=1) as consts,
            tc.tile_pool(name="xp", bufs=x_bufs) as xp,
            tc.tile_pool(name="outp", bufs=6) as outp,
            tc.tile_pool(name="psum", bufs=psum_bufs, space="PSUM") as psump,
        ):
            # W/alpha/bias loads go through the scalar engine's HWDGE queue so
            # the x-tile stream (sync queue) isn't stuck behind the 8MB weight
            # load at kernel start.
            a_sb = consts.tile([P, o_shard], w_in_dt)
            nc.scalar.dma_start(out=a_sb[:], in_=a_rep[:])
            b_sb = consts.tile([P, o_shard], f32)
            nc.scalar.dma_start(out=b_sb[:], in_=b_rep[:])

            # bw = (2 * (W >= 0) - 1) * alpha, computed in fp32 (exact),
            # final multiply writes the matmul-dtype copy.
            if mm_dt == f32 or variant == "f32r":
                W_mm = consts.tile([P, KO, o_shard], mm_dt)
                W_f32 = W_mm  # binarize in place (f32r is fp32 storage)
            else:
                W_f32 = consts.tile([P, KO, o_shard], f32)
                W_mm = consts.tile([P, KO, o_shard], mm_dt)
            # Pipeline the binarize across engines (gpsimd does the compare,
            # vector the affine + alpha scale) so W_mm k-tiles are produced
            # faster than the first chunk's matmuls consume them.
            for ko in range(KO):
                w2d = W_f32[:, ko]
                nc.scalar.dma_start(out=w2d, in_=WT_t[:, ko])
                nc.gpsimd.tensor_scalar(
                    w2d, w2d, 0.0, 2.0, mybir.AluOpType.is_ge, mybir.AluOpType.mult
                )
                nc.vector.tensor_scalar(
                    w2d, w2d, 1.0, None, mybir.AluOpType.subtract
                )
                nc.vector.tensor_tensor(
                    W_mm[:, ko], w2d, a_sb[:], mybir.AluOpType.mult
                )

            for nch in range(NCH):
                psums = [
                    psump.tile([P, o_shard], f32, tag=f"ps{ns}", name=f"ps{ns}")
                    for ns in range(NS)
                ]
                for k in range(KO):
                    x_t = xp.tile([P, n_chunk], x_dt, tag="xt")
                    nc.sync.dma_start(
                        out=x_t[:],
                        in_=xT_t[k, :, nch * n_chunk : (nch + 1) * n_chunk],
                    )
                    for ns in range(NS):
                        nc.tensor.matmul(
                            psums[ns][:],
                            x_t[:, ns * P : (ns + 1) * P],
                            W_mm[:, k],
                            start=(k == 0),
                            stop=(k == KO - 1),
                        )
                for ns in range(NS):
                    o_sb = outp.tile([P, o_shard], f32, tag="o")
                    nc.vector.tensor_tensor(
                        o_sb[:], psums[ns][:], b_sb[:], mybir.AluOpType.add
                    )
                    row0 = nch * n_chunk + ns * P
                    nc.sync.dma_start(
                        out=out[row0 : row0 + P, :], in_=o_sb[:]
                    )
    nc.compile()
    return nc


def make_in_maps(x, W, alpha, b, n_cores=N_CORES, variant=VARIANT):
    """Shard full inputs into per-core input maps (host-side relayout only)."""
    o_shard = W.shape[0] // n_cores
    xT = np.ascontiguousarray(x.T)
    if variant == "bf16":
        import ml_dtypes

        xT = xT.astype(ml_dtypes.bfloat16)
    in_maps = []
    for c in range(n_cores):
        sl = slice(c * o_shard, (c + 1) * o_shard)
        in_maps.append(
            {
                "xT": xT,
                "WT": np.ascontiguousarray(W[sl].T),
                "a_rep": np.ascontiguousarray(
                    np.broadcast_to(alpha[sl].reshape(1, -1), (P, o_shard)),
                    dtype=np.float32,
                ),
                "b_rep": np.ascontiguousarray(
                    np.broadcast_to(b[sl].reshape(1, -1), (P, o_shard)),
                    dtype=np.float32,
                ),
            }
        )
    return in_maps


_NC_CACHE = {}


def kernel(x, W, alpha, b, trace=False, variant=VARIANT):
    x = np.asarray(x, dtype=np.float32)
    W = np.asarray(W, dtype=np.float32)
    alpha = np.asarray(alpha, dtype=np.float32)
    b = np.asarray(b, dtype=np.float32)

    n_rows, in_f = x.shape
    out_f = W.shape[0]
    o_shard = out_f // N_CORES

    key = (n_rows, in_f, o_shard, variant)
    if key not in _NC_CACHE:
        _NC_CACHE[key] = build_nc(
            n_rows=n_rows, in_f=in_f, o_shard=o_shard, variant=variant
        )
    nc = _NC_CACHE[key]

    in_maps = make_in_maps(x, W, alpha, b, variant=variant)
    res = run_bass_kernel_spmd(
        nc, in_maps, core_ids=list(range(N_CORES)), trace=trace
    )
    shards = [np.asarray(res.results[c]["out"]) for c in range(N_CORES)]
    full = np.concatenate(shards, axis=1).astype(np.float32)
    if trace:
        return full, res
    return full


if __name__ == "__main__":
    for v in ("f32", "f32r", "bf16"):
        nc = build_nc(n_rows=512, in_f=512, o_shard=256, variant=v, n_chunk=256)
        print(f"build ok [{v}]")


# revision 19
# speedup vs baseline: 1.4343x; 1.4343x over previous
"""BinaryLinear (8192x4096 @ 4096x4096 binarized) on 8 TRN2 NeuronCores.

Strategy (tensor-parallel, column sharding per out_features):
  - Shard W/alpha/b along out_features: each core gets 512 output channels.
  - Replicate x (host pre-transposed to [in_f, n_rows] so the contraction
    dim lands on SBUF partitions without any device-side transpose).
  - Per core: out_shard[n, o] = sum_k xT[k, n] * bwT[k, o] + b[o], where
    bw = sign(W) * alpha is computed on device in fp32 (exact match of
    jnp.where(W >= 0, 1, -1) * alpha), then cast to the matmul dtype.
  - Host gathers the 8 [8192, 512] shards with a concatenate on axis 1.

Matmul layout per core:
  lhsT = x tile [K=128, M=128] (stationary), rhs = bwT tile [K=128, N=512]
  (moving), accumulating over 32 K-tiles into a [128, 512] PSUM bank.

Variants:
  f32    - full-precision fp32 matmul (4 cyc/row), reference-grade
  f32r   - fp32 storage, reduced-precision PE mode (~1e-4 rel err)
  bf16   - x shipped as bf16 (halves x DMA), weights binarized on device
           then cast to bf16 (~2e-3 rel err, fastest)
"""

import os
import sys

sys.path.insert(0, "/opt/trn_rl_repo")

import numpy as np

from concourse import bacc, bass, mybir
import concourse.tile as tile
from concourse.bass_utils import run_bass_kernel_spmd

N_ROWS = 8192
IN_F = 4096
OUT_F = 4096
N_CORES = 8
O_SHARD = OUT_F // N_CORES  # 512

P = 128

VARIANT = "bf16"  # f32 | f32r | bf16


def build_nc(
    n_rows=N_ROWS,
    in_f=IN_F,
    o_shard=O_SHARD,
    variant=VARIANT,
    n_chunk=None,
    x_bufs=16,
):
    """Build the per-core Bass graph (same program on all cores, SPMD)."""
    f32 = mybir.dt.float32
    if variant == "f32":
        x_dt = mm_dt = f32
    elif variant == "f32r":
        x_dt = mm_dt = mybir.dt.float32r
    elif variant == "bf16":
        x_dt = mm_dt = mybir.dt.bfloat16
    else:
        raise ValueError(variant)
    if n_chunk is None:
        n_chunk = 512

    assert in_f % P == 0 and n_rows % n_chunk == 0 and n_chunk % P == 0
    assert o_shard <= 512  # one PSUM bank per psum tile (fp32 out)
    KO = in_f // P
    NCH = n_rows // n_chunk
    NS = n_chunk // P
    assert NS * 2 <= 16  # psum tags * bufs fit in 8 banks w/ bufs>=1

    nc = bacc.Bacc("TRN2", target_bir_lowering=False)

    # f32r is fp32 storage; type the whole W/alpha producer chain f32r so the
    # BIR verifier's checkMatmultFP32r accepts the matmul inputs.
    w_in_dt = mm_dt if variant == "f32r" else f32
    xT = nc.declare_dram_parameter("xT", [in_f, n_rows], x_dt, isOutput=False)
    WT = nc.declare_dram_parameter("WT", [in_f, o_shard], w_in_dt, isOutput=False)
    a_rep = nc.declare_dram_parameter("a_rep", [P, o_shard], w_in_dt, isOutput=False)
    b_rep = nc.declare_dram_parameter("b_rep", [P, o_shard], f32, isOutput=False)
    out = nc.declare_dram_parameter("out", [n_rows, o_shard], f32, isOutput=True)

    xT_t = xT[:].rearrange("(ko p) n -> ko p n", p=P)
    WT_t = WT[:].rearrange("(ko p) o -> p ko o", p=P)

    psum_bufs = 2 if NS * 2 <= 8 else 1

    with tile.TileContext(nc) as tc:
        with (
            tc.tile_pool(name="consts", bufs=1) as consts,
            tc.tile_pool(name="xp", bufs=x_bufs) as xp,
            tc.tile_pool(name="outp", bufs=6) as outp,
            tc.tile_pool(name="psum", bufs=psum_bufs, space="PSUM") as psump,
        ):
            # W/alpha/bias loads go through the scalar engine's HWDGE queue so
            # the x-tile stream (sync queue) isn't stuck behind the 8MB weight
            # load at kernel start.
            a_sb = consts.tile([P, o_shard], w_in_dt)
            nc.scalar.dma_start(out=a_sb[:], in_=a_rep[:])
            b_sb = consts.tile([P, o_shard], f32)
            nc.scalar.dma_start(out=b_sb[:], in_=b_rep[:])

            # bw = (2 * (W >= 0) - 1) * alpha, computed in fp32 (exact),
            # final multiply writes the matmul-dtype copy.
            if mm_dt == f32 or variant == "f32r":
                W_mm = consts.tile([P, KO, o_shard], mm_dt)
                W_f32 = W_mm  # binarize in place (f32r is fp32 storage)
            else:
                W_f32 = consts.tile([P, KO, o_shard], f32)
                W_mm = consts.tile([P, KO, o_shard], mm_dt)
            for ko in range(KO):
                w2d = W_f32[:, ko]
                nc.scalar.dma_start(out=w2d, in_=WT_t[:, ko])
                nc.vector.tensor_scalar(
                    w2d, w2d, 0.0, 2.0, mybir.AluOpType.is_ge, mybir.AluOpType.mult
                )
                nc.vector.tensor_scalar(
                    w2d, w2d, 1.0, None, mybir.AluOpType.subtract
                )
                nc.vector.tensor_tensor(
                    W_mm[:, ko], w2d, a_sb[:], mybir.AluOpType.mult
                )

            for nch in range(NCH):
                psums = [
                    psump.tile([P, o_shard], f32, tag=f"ps{ns}", name=f"ps{ns}")
                    for ns in range(NS)
                ]
                for k in range(KO):
                    x_t = xp.tile([P, n_chunk], x_dt, tag="xt")
                    nc.sync.dma_start(
                        out=x_t[:],
                        in_=xT_t[k, :, nch * n_chunk : (nch + 1) * n_chunk],
                    )
                    for ns in range(NS):
                        nc.tensor.matmul(
                            psums[ns][:],
                            x_t[:, ns * P : (ns + 1) * P],
                            W_mm[:, k],
                            start=(k == 0),
                            stop=(k == KO - 1),
                        )
                for ns in range(NS):
                    o_sb = outp.tile([P, o_shard], f32, tag="o")
                    nc.vector.tensor_tensor(
                        o_sb[:], psums[ns][:], b_sb[:], mybir.AluOpType.add
                    )
                    row0 = nch * n_chunk + ns * P
                    nc.sync.dma_start(
                        out=out[row0 : row0 + P, :], in_=o_sb[:]
                    )
    nc.compile()
    return nc


def make_in_maps(x, W, alpha, b, n_cores=N_CORES, variant=VARIANT):
    """Shard full inputs into per-core input maps (host-side relayout only)."""
    o_shard = W.shape[0] // n_cores
    xT = np.ascontiguousarray(x.T)
    if variant == "bf16":
        import ml_dtypes

        xT = xT.astype(ml_dtypes.bfloat16)
    in_maps = []
    for c in range(n_cores):
        sl = slice(c * o_shard, (c + 1) * o_shard)
        in_maps.append(
            {
                "xT": xT,
                "WT": np.ascontiguousarray(W[sl].T),
                "a_rep": np.ascontiguousarray(
                    np.broadcast_to(alpha[sl].reshape(1, -1), (P, o_shard)),
                    dtype=np.float32,
                ),
                "b_rep": np.ascontiguousarray(
                    np.broadcast_to(b[sl].reshape(1, -1), (P, o_shard)),
                    dtype=np.float32,
                ),
            }
        )
    return in_maps


_NC_CACHE = {}


def kernel(x, W, alpha, b, trace=False, variant=VARIANT):
    x = np.asarray(x, dtype=np.float32)
    W = np.asarray(W, dtype=np.float32)
    alpha = np.asarray(alpha, dtype=np.float32)
    b = np.asarray(b, dtype=np.float32)

    n_rows, in_f = x.shape
    out_f = W.shape[0]
    o_shard = out_f // N_CORES

    key = (n_rows, in_f, o_shard, variant)
    if key not in _NC_CACHE:
        _NC_CACHE[key] = build_nc(
            n_rows=n_rows, in_f=in_f, o_shard=o_shard, variant=variant
        )
    nc = _NC_CACHE[key]

    in_maps = make_in_maps(x, W, alpha, b, variant=variant)
    res = run_bass_kernel_spmd(
        nc, in_maps, core_ids=list(range(N_CORES)), trace=trace
    )
    shards = [np.asarray(res.results[c]["out"]) for c in range(N_CORES)]
    full = np.concatenate(shards, axis=1).astype(np.float32)
    if trace:
        return full, res
    return full


if __name__ == "__main__":
    for v in ("f32", "f32r", "bf16"):
        nc = build_nc(n_rows=512, in_f=512, o_shard=256, variant=v, n_chunk=256)
        print(f"build ok [{v}]")
